# revision 10
# baseline (speedup 1.0000x reference)
"""GCN + batch-attention kernel for Trainium2 (8 NeuronCores, SPMD).

Problem (nn_GCNResnet): for x [8192,3,10], A [3,3], W [10,10]:
    adj   = 0.25*(off_diag_ones + A)                    # normalized adjacency
    pooled= 0.5*(h0+h1),  h = adj @ (x @ W)             # -> [B,10]
    v     = softmax(pooled @ pooled.T) @ pooled         # -> [B,10]

Formulation: pooled = x2 @ Wcp with x2 = x.reshape(B,30) padded to 32 features
(col 30 = ones, col 31 = 0) and Wcp the host-folded [32,10] weight. Scores
s_ij = pooled_i . pooled_j = x2_i^T G x2_j with G = Wcp Wcp^T [32,32], so the
kernel never materializes pooledT: S^T chunks come straight from x2T slices
(lhsT) against Gx2 = G @ x2T_local (rhs).

Host prep (pure layout/folding, no attention FLOPs): x2 is rolled per core,
cast to fp16 and sent already transposed as xt [32,8192]; Wc [32,12] has a
ones-selector col 10 (augmented-V ones column -> softmax denominator) and a
zero col 11; G is fp16.

Per core i (batch-sharded attention; input rolled by 1024*i rows so the
identical SPMD program always works on local query rows 0:1024):
  - Gx2 [32,1024] = G @ xt[:,0:1024] on PE, copied to SBUF fp16 (halves, so
    the first S matmul starts after half a copy)
  - vn [128,8g+j,12] = natural [pooled|1|0] rows, one 8-matmul group + one
    [128,96] copy per 1024-row group, interleaved with the chunk stream
  - flash attention, never materializing the [B,B] score matrix: for each of
    64 kv chunks, S^T [128,1024] = xt_chunk^T @ Gx2 (2 fp16 matmuls, fp32
    PSUM), then exp:
      * ACT chunks: native Exp -> fp16 E
      * DVE chunks: Schraudolph bit-trick in one tensor_scalar:
        int16(s*2^10/ln2 + (15*2^10 - 50)) bitcast to fp16 (~1% err; softmax
        normalization cancels most of it -- end-to-end ~2e-3 rel err)
    The ACT/DVE split is tuned so both engines drain the PSUM score stream
    in parallel (PSUM egress is the throughput wall: only ACT+DVE can read
    PSUM -- gpsimd and DMA cannot).
  - PV with E stationary: per chunk 8 matmuls out[128,12] += E[:,u::8]^T @
    vn_chunk accumulate in PSUM over all 64 chunks (N=12 fp16 at full rate,
    ~24x cheaper on PE than the vn-stationary orientation). The u::8 column
    stride makes partition p own query rows 8p..8p+7, so the output DMA
    writes one contiguous 320B line per partition.
  - epilogue: v = acc[:,0:10] * (1/acc[:,10]) on DVE, single DMA out.
"""

import math

import numpy as np

import concourse.bass as bass
import concourse.bacc as bacc
import concourse.mybir as mybir
import concourse.tile as tile
from concourse.bass_utils import run_bass_kernel_spmd

B = 8192
NCORES = 8
QL = B // NCORES          # 1024 local query rows
NF = 32                   # 30 feats + ones + zero pad
D = 10
DV = 12                   # [pooled | 1 | 0]
NG = 8                    # groups of 1024 batch rows
NKV = B // 128            # 64 kv chunks

f32 = mybir.dt.float32
f16 = mybir.dt.float16
i16 = mybir.dt.int16
EXP = mybir.ActivationFunctionType.Exp

# fp16 Schraudolph: exp(s) ~= bitcast_f16(int16(s*2^10/ln2 + 15*2^10 - CC));
# CC centers the piecewise-linear 2^frac error (tuned end-to-end on host).
C1 = 1024.0 / math.log(2.0)
CC = 50.0
C2 = 15.0 * 1024.0 - CC

N_DVE = 27                # kv chunks on the DVE bit-trick path (rest: ACT)

_NC = None


def _dve_mask():
    m = [((c + 1) * N_DVE) // NKV > (c * N_DVE) // NKV for c in range(NKV)]
    # drain: last chunk on ACT (shorter op), second-to-last on DVE
    if m[NKV - 1]:
        m[NKV - 1], m[NKV - 2] = False, True
    return m


def _build():
    nc = bacc.Bacc(trn_type="TRN2", target_bir_lowering=False)

    xt = nc.dram_tensor("xt", [NF, B], f16, kind="ExternalInput")
    gw = nc.dram_tensor("gw", [NF, NF + DV], f16, kind="ExternalInput")
    v = nc.dram_tensor("v", [QL, D], f32, kind="ExternalOutput")

    dve = _dve_mask()

    with tile.TileContext(nc) as tc:
        with (
            tc.tile_pool(name="const", bufs=1) as const,
            tc.tile_pool(name="xts", bufs=1) as xts,
            tc.tile_pool(name="vns", bufs=1) as vns,
            tc.tile_pool(name="epool", bufs=5) as epool,
            tc.tile_pool(name="outp", bufs=1) as outp,
            tc.tile_pool(name="psb", bufs=3, space="PSUM") as psb,
            tc.tile_pool(name="psv", bufs=1, space="PSUM") as psv,
            tc.tile_pool(name="psp", bufs=1, space="PSUM") as psp,
        ):
            gw_sb = const.tile([NF, NF + DV], f16, tag="gw")
            g_sb = gw_sb[:, 0:NF]
            wc_sb = gw_sb[:, NF:NF + DV]
            xt_sb = xts.tile([NF, B], f16, tag="xt")
            gx = const.tile([NF, QL], f16, tag="gx")
            vn = vns.tile([128, NKV, DV], f16, tag="vn")

            # gw heads the ACT hwdge queue (before the Exp table load), xt
            # local halves on SP: both input chains to the first S matmul
            # resolve in parallel at ~2.5us
            nc.scalar.dma_start(out=gw_sb[:, :], in_=gw[:, :])

            # PE warm-up with no DMA dependency (starts the p-state ramp clock)
            # plus one real Exp to pull LoadActFuncSet off the critical path.
            wz = const.tile([128, 128], f32, tag="wz")
            nc.vector.memset(wz[:, :], 0.0)
            actwarm = const.tile([2, 2], f16, tag="actwarm")
            nc.scalar.activation(out=actwarm[:, :], in_=wz[0:2, 0:2], func=EXP)
            warm = psb.tile([128, 1024], f32, tag="ps")
            for _ in range(2):
                nc.tensor.matmul(
                    warm[:, 0:64], wz[:, :], wz[:, 0:64],
                    start=True, stop=True,
                )

            nc.sync.dma_start(out=xt_sb[:, 0:512], in_=xt[:, 0:512])
            nc.sync.dma_start(out=xt_sb[:, 512:QL], in_=xt[:, 512:QL])
            for g in range(1, NG):
                nc.sync.dma_start(
                    out=xt_sb[:, QL * g:QL * (g + 1)],
                    in_=xt[:, QL * g:QL * (g + 1)])

            # Gx2 = G @ xt_local, half at a time so S(c0,h0) starts early
            gps = psb.tile([NF, QL], f32, tag="ps")
            for h in range(2):
                nc.tensor.matmul(
                    gps[:, 512 * h:512 * (h + 1)], g_sb,
                    xt_sb[:, 512 * h:512 * (h + 1)], start=True, stop=True,
                )
                nc.vector.tensor_copy(
                    gx[:, 512 * h:512 * (h + 1)],
                    gps[:, 512 * h:512 * (h + 1)])

            # double-buffered vn staging inside ONE psum bank: halves alternate
            # by group parity, so PE's vn(g+1) never waits on the copy of
            # vn(g) (a bufs=1 pool slot would serialize PE behind DVE here).
            vnp = psv.tile([128, 2, NG, DV], f32, tag="psv")

            def emit_vn(g):
                h = vnp[:, g % 2, :, :]
                for j in range(NG):
                    nc.tensor.matmul(
                        h[:, j, :],
                        xt_sb[:, QL * g + 128 * j:QL * g + 128 * (j + 1)],
                        wc_sb, start=(j == 0), stop=(j == NG - 1),
                    )
                nc.vector.tensor_copy(vn[:, NG * g:NG * (g + 1), :], h)

            def emit_s(c):
                st = psb.tile([128, QL], f32, tag="ps")
                lhs = xt_sb[:, 128 * c:128 * (c + 1)]
                for h in range(2):
                    nc.tensor.matmul(
                        st[:, 512 * h:512 * (h + 1)], lhs,
                        gx[:, 512 * h:512 * (h + 1)], start=True, stop=True,
                    )
                return st

            def emit_exp(c, st):
                et = epool.tile([128, QL], i16, tag="E")
                ev = et[:, :].bitcast(f16)
                if dve[c]:
                    nc.vector.tensor_scalar(
                        out=et[:, :], in0=st[:, :], scalar1=C1, scalar2=C2,
                        op0=mybir.AluOpType.mult, op1=mybir.AluOpType.add)
                elif c == 0:
                    # halves: first exp starts half a matmul+copy earlier
                    for h in range(2):
                        nc.scalar.activation(
                            out=ev[:, 512 * h:512 * (h + 1)],
                            in_=st[:, 512 * h:512 * (h + 1)], func=EXP)
                else:
                    nc.scalar.activation(out=ev[:, :], in_=st[:, :], func=EXP)
                return ev

            pv = psp.tile([128, NG, DV], f32, tag="psp")

            def emit_pv(c, ev):
                rhs = vn[:, c, :]
                for u in range(NG):
                    lhs = bass.AP(ev.tensor, ev.offset + u, [ev.ap[0], [8, 128]])
                    nc.tensor.matmul(
                        pv[:, u, :], lhs, rhs,
                        start=(c == 0 and u == 0),
                        stop=(c == NKV - 1 and u == NG - 1),
                    )

            # chunk stream; vn(g) one group ahead at c%8==6; PV trails exp by
            # 2 chunks so PE's waiting PV matmuls never block S production.
            LOOK = 3
            e_tiles = {}
            s0 = emit_s(0)
            e_tiles[0] = emit_exp(0, s0)
            emit_vn(0)
            for c in range(1, NKV):
                g_next = c // NG + 1
                if c % NG == 6 and g_next < NG:
                    emit_vn(g_next)
                st = emit_s(c)
                e_tiles[c] = emit_exp(c, st)
                if c - LOOK >= 1:
                    emit_pv(c - LOOK, e_tiles.pop(c - LOOK))
                elif c == LOOK:
                    emit_pv(0, e_tiles.pop(0))
            for c in range(NKV - LOOK, NKV):
                emit_pv(c, e_tiles.pop(c))

            # ---- epilogue: v = acc[:,0:10] / acc[:,10] ----
            rec = outp.tile([128, NG], f32, tag="rec")
            nc.vector.reciprocal(rec[:, :], pv[:, :, D])
            vout = outp.tile([128, NG, D], f32, tag="vout")
            rec_b = bass.AP(rec[:, :].tensor, rec[:, :].offset,
                            [rec[:, :].ap[0], [1, NG], [0, D]])
            nc.vector.tensor_mul(vout[:, :, :], pv[:, :, 0:D], rec_b)
            dst = bass.AP(v, 0, [[NG * D, 128], [1, NG * D]])
            nc.sync.dma_start(
                out=dst, in_=vout[:, :, :].rearrange("p j d -> p (j d)"))

    nc.finalize()
    return nc


def _get_nc():
    global _NC
    if _NC is None:
        _NC = _build()
    return _NC


def _host_prep(x, A, W):
    """Fold adjacency normalization + node pooling into Wc [32,12]; build
    padded x2 [B,32] (col 30 = ones) and the score kernel G = Wcp Wcp^T."""
    A = np.asarray(A, np.float32)
    W = np.asarray(W, np.float32)
    off = np.ones((3, 3), np.float32) - np.eye(3, dtype=np.float32)
    d = 0.5 * np.eye(3, dtype=np.float32)
    adj = (d @ (off + A) @ d).astype(np.float32)
    c = (0.5 * (adj[0, :] + adj[1, :])).astype(np.float32)
    wc = np.zeros((NF, DV), np.float32)
    wc[0:30, 0:D] = np.einsum("n,fo->nfo", c, W).reshape(30, D)
    wc[30, D] = 1.0
    x2 = np.zeros((B, NF), np.float32)
    x2[:, 0:30] = np.asarray(x, np.float32).reshape(B, 30)
    x2[:, 30] = 1.0
    wcp = wc[:, 0:D]
    g = (wcp @ wcp.T).astype(np.float32)
    gwm = np.concatenate([g, wc], axis=1).astype(np.float16)
    return x2.astype(np.float16), gwm


def kernel(x, A, W):
    x2f, gwm = _host_prep(x, A, W)
    nc = _get_nc()
    in_maps = [
        {"xt": np.ascontiguousarray(np.roll(x2f, -QL * i, axis=0).T),
         "gw": gwm}
        for i in range(NCORES)
    ]
    res = run_bass_kernel_spmd(nc, in_maps, core_ids=list(range(NCORES)))
    return np.concatenate([res.results[i]["v"] for i in range(NCORES)], axis=0)


# revision 12
# speedup vs baseline: 1.0112x; 1.0112x over previous
"""GCN + batch-attention kernel for Trainium2 (8 NeuronCores, SPMD).

Problem (nn_GCNResnet): for x [8192,3,10], A [3,3], W [10,10]:
    adj   = 0.25*(off_diag_ones + A)                    # normalized adjacency
    pooled= 0.5*(h0+h1),  h = adj @ (x @ W)             # -> [B,10]
    v     = softmax(pooled @ pooled.T) @ pooled         # -> [B,10]

Formulation: pooled = x2 @ Wcp with x2 = x.reshape(B,30) padded to 32 features
(col 30 = ones, col 31 = 0) and Wcp the host-folded [32,10] weight. Scores
s_ij = pooled_i . pooled_j = x2_i^T G x2_j with G = Wcp Wcp^T [32,32], so the
kernel never materializes pooledT: S^T chunks come straight from x2T slices
(lhsT) against Gx2 = G @ x2T_local (rhs).

Host prep (pure layout/folding, no attention FLOPs): x2 is rolled per core,
cast to fp16 and sent already transposed as xt [32,8192]; Wc [32,12] has a
ones-selector col 10 (augmented-V ones column -> softmax denominator) and a
zero col 11; G is fp16.

Per core i (batch-sharded attention; input rolled by 1024*i rows so the
identical SPMD program always works on local query rows 0:1024):
  - Gx2 [32,1024] = G @ xt[:,0:1024] on PE, copied to SBUF fp16 (halves, so
    the first S matmul starts after half a copy)
  - vn [128,8g+j,12] = natural [pooled|1|0] rows, one 8-matmul group + one
    [128,96] copy per 1024-row group, interleaved with the chunk stream
  - flash attention, never materializing the [B,B] score matrix: for each of
    64 kv chunks, S^T [128,1024] = xt_chunk^T @ Gx2 (2 fp16 matmuls, fp32
    PSUM), then exp:
      * ACT chunks: native Exp -> fp16 E
      * DVE chunks: Schraudolph bit-trick in one tensor_scalar:
        int16(s*2^10/ln2 + (15*2^10 - 50)) bitcast to fp16 (~1% err; softmax
        normalization cancels most of it -- end-to-end ~2e-3 rel err)
    The ACT/DVE split is tuned so both engines drain the PSUM score stream
    in parallel (PSUM egress is the throughput wall: only ACT+DVE can read
    PSUM -- gpsimd and DMA cannot).
  - PV with E stationary: per chunk 8 matmuls out[128,12] += E[:,u::8]^T @
    vn_chunk accumulate in PSUM over all 64 chunks (N=12 fp16 at full rate,
    ~24x cheaper on PE than the vn-stationary orientation). The u::8 column
    stride makes partition p own query rows 8p..8p+7, so the output DMA
    writes one contiguous 320B line per partition.
  - epilogue: v = acc[:,0:10] * (1/acc[:,10]) on DVE, single DMA out.
"""

import math

import numpy as np

import concourse.bass as bass
import concourse.bacc as bacc
import concourse.mybir as mybir
import concourse.tile as tile
from concourse.bass_utils import run_bass_kernel_spmd

B = 8192
NCORES = 8
QL = B // NCORES          # 1024 local query rows
NF = 32                   # 30 feats + ones + zero pad
D = 10
DV = 12                   # [pooled | 1 | 0]
NG = 8                    # groups of 1024 batch rows
NKV = B // 128            # 64 kv chunks

f32 = mybir.dt.float32
f16 = mybir.dt.float16
i16 = mybir.dt.int16
EXP = mybir.ActivationFunctionType.Exp

# fp16 Schraudolph: exp(s) ~= bitcast_f16(int16(s*2^10/ln2 + 15*2^10 - CC));
# CC centers the piecewise-linear 2^frac error (tuned end-to-end on host).
C1 = 1024.0 / math.log(2.0)
CC = 50.0
C2 = 15.0 * 1024.0 - CC

N_DVE = 27                # kv chunks on the DVE bit-trick path (rest: ACT)

_NC = None


def _dve_mask():
    m = [((c + 1) * N_DVE) // NKV > (c * N_DVE) // NKV for c in range(NKV)]
    # drain: last chunk on ACT (shorter op), second-to-last on DVE
    if m[NKV - 1]:
        m[NKV - 1], m[NKV - 2] = False, True
    return m


def _build():
    nc = bacc.Bacc(trn_type="TRN2", target_bir_lowering=False)

    xt = nc.dram_tensor("xt", [NF, B], f16, kind="ExternalInput")
    gw = nc.dram_tensor("gw", [NF, NF + DV], f16, kind="ExternalInput")
    v = nc.dram_tensor("v", [QL, D], f32, kind="ExternalOutput")

    dve = _dve_mask()

    with tile.TileContext(nc) as tc:
        with (
            tc.tile_pool(name="const", bufs=1) as const,
            tc.tile_pool(name="xts", bufs=1) as xts,
            tc.tile_pool(name="vns", bufs=1) as vns,
            tc.tile_pool(name="epool", bufs=5) as epool,
            tc.tile_pool(name="outp", bufs=1) as outp,
            tc.tile_pool(name="psb", bufs=3, space="PSUM") as psb,
            tc.tile_pool(name="psv", bufs=1, space="PSUM") as psv,
            tc.tile_pool(name="psp", bufs=1, space="PSUM") as psp,
        ):
            gw_sb = const.tile([NF, NF + DV], f16, tag="gw")
            g_sb = gw_sb[:, 0:NF]
            wc_sb = gw_sb[:, NF:NF + DV]
            xt_sb = xts.tile([NF, B], f16, tag="xt")
            gx = const.tile([NF, QL], f16, tag="gx")
            vn = vns.tile([128, NKV, DV], f16, tag="vn")

            # gw + first local xt half on SP, second half on the ACT hwdge
            # queue: the input chains to the first S matmul resolve in
            # parallel at ~2.6us
            nc.sync.dma_start(out=gw_sb[:, :], in_=gw[:, :])
            nc.sync.dma_start(out=xt_sb[:, 0:512], in_=xt[:, 0:512])
            nc.scalar.dma_start(out=xt_sb[:, 512:QL], in_=xt[:, 512:QL])

            # PE warm-up with no DMA dependency (starts the p-state ramp clock)
            # plus one real Exp to pull LoadActFuncSet off the critical path.
            wz = const.tile([128, 128], f32, tag="wz")
            nc.vector.memset(wz[:, :], 0.0)
            actwarm = const.tile([2, 2], f16, tag="actwarm")
            nc.scalar.activation(out=actwarm[:, :], in_=wz[0:2, 0:2], func=EXP)
            warm = psb.tile([128, 1024], f32, tag="ps")
            for _ in range(2):
                nc.tensor.matmul(
                    warm[:, 0:64], wz[:, :], wz[:, 0:64],
                    start=True, stop=True,
                )

            for g in range(1, NG):
                nc.sync.dma_start(
                    out=xt_sb[:, QL * g:QL * (g + 1)],
                    in_=xt[:, QL * g:QL * (g + 1)])

            # Gx2 = G @ xt_local, half at a time so S(c0,h0) starts early
            gps = psb.tile([NF, QL], f32, tag="ps")
            for h in range(2):
                nc.tensor.matmul(
                    gps[:, 512 * h:512 * (h + 1)], g_sb,
                    xt_sb[:, 512 * h:512 * (h + 1)], start=True, stop=True,
                )
                nc.vector.tensor_copy(
                    gx[:, 512 * h:512 * (h + 1)],
                    gps[:, 512 * h:512 * (h + 1)])

            # double-buffered vn staging inside ONE psum bank: halves alternate
            # by group parity, so PE's vn(g+1) never waits on the copy of
            # vn(g) (a bufs=1 pool slot would serialize PE behind DVE here).
            vnp = psv.tile([128, 2, NG, DV], f32, tag="psv")

            def emit_vn(g):
                h = vnp[:, g % 2, :, :]
                for j in range(NG):
                    nc.tensor.matmul(
                        h[:, j, :],
                        xt_sb[:, QL * g + 128 * j:QL * g + 128 * (j + 1)],
                        wc_sb, start=(j == 0), stop=(j == NG - 1),
                    )
                nc.vector.tensor_copy(vn[:, NG * g:NG * (g + 1), :], h)

            def emit_s(c):
                st = psb.tile([128, QL], f32, tag="ps")
                lhs = xt_sb[:, 128 * c:128 * (c + 1)]
                for h in range(2):
                    nc.tensor.matmul(
                        st[:, 512 * h:512 * (h + 1)], lhs,
                        gx[:, 512 * h:512 * (h + 1)], start=True, stop=True,
                    )
                return st

            def emit_exp(c, st):
                et = epool.tile([128, QL], i16, tag="E")
                ev = et[:, :].bitcast(f16)
                if dve[c]:
                    nc.vector.tensor_scalar(
                        out=et[:, :], in0=st[:, :], scalar1=C1, scalar2=C2,
                        op0=mybir.AluOpType.mult, op1=mybir.AluOpType.add)
                elif c == 0:
                    # halves: first exp starts half a matmul+copy earlier
                    for h in range(2):
                        nc.scalar.activation(
                            out=ev[:, 512 * h:512 * (h + 1)],
                            in_=st[:, 512 * h:512 * (h + 1)], func=EXP)
                else:
                    nc.scalar.activation(out=ev[:, :], in_=st[:, :], func=EXP)
                return ev

            pv = psp.tile([128, NG, DV], f32, tag="psp")

            def emit_pv(c, ev):
                rhs = vn[:, c, :]
                for u in range(NG):
                    lhs = bass.AP(ev.tensor, ev.offset + u, [ev.ap[0], [8, 128]])
                    nc.tensor.matmul(
                        pv[:, u, :], lhs, rhs,
                        start=(c == 0 and u == 0),
                        stop=(c == NKV - 1 and u == NG - 1),
                    )

            # chunk stream; vn(g) one group ahead at c%8==6; PV trails exp by
            # 2 chunks so PE's waiting PV matmuls never block S production.
            LOOK = 3
            e_tiles = {}
            s0 = emit_s(0)
            e_tiles[0] = emit_exp(0, s0)
            emit_vn(0)
            for c in range(1, NKV):
                g_next = c // NG + 1
                if c % NG == 6 and g_next < NG:
                    emit_vn(g_next)
                st = emit_s(c)
                e_tiles[c] = emit_exp(c, st)
                if c - LOOK >= 1:
                    emit_pv(c - LOOK, e_tiles.pop(c - LOOK))
                elif c == LOOK:
                    emit_pv(0, e_tiles.pop(0))
            for c in range(NKV - LOOK, NKV):
                emit_pv(c, e_tiles.pop(c))

            # ---- epilogue: v = acc[:,0:10] / acc[:,10] ----
            rec = outp.tile([128, NG], f32, tag="rec")
            nc.vector.reciprocal(rec[:, :], pv[:, :, D])
            vout = outp.tile([128, NG, D], f32, tag="vout")
            rec_b = bass.AP(rec[:, :].tensor, rec[:, :].offset,
                            [rec[:, :].ap[0], [1, NG], [0, D]])
            nc.vector.tensor_mul(vout[:, :, :], pv[:, :, 0:D], rec_b)
            dst = bass.AP(v, 0, [[NG * D, 128], [1, NG * D]])
            nc.sync.dma_start(
                out=dst, in_=vout[:, :, :].rearrange("p j d -> p (j d)"))

    nc.finalize()
    return nc


def _get_nc():
    global _NC
    if _NC is None:
        _NC = _build()
    return _NC


def _host_prep(x, A, W):
    """Fold adjacency normalization + node pooling into Wc [32,12]; build
    padded x2 [B,32] (col 30 = ones) and the score kernel G = Wcp Wcp^T."""
    A = np.asarray(A, np.float32)
    W = np.asarray(W, np.float32)
    off = np.ones((3, 3), np.float32) - np.eye(3, dtype=np.float32)
    d = 0.5 * np.eye(3, dtype=np.float32)
    adj = (d @ (off + A) @ d).astype(np.float32)
    c = (0.5 * (adj[0, :] + adj[1, :])).astype(np.float32)
    wc = np.zeros((NF, DV), np.float32)
    wc[0:30, 0:D] = np.einsum("n,fo->nfo", c, W).reshape(30, D)
    wc[30, D] = 1.0
    x2 = np.zeros((B, NF), np.float32)
    x2[:, 0:30] = np.asarray(x, np.float32).reshape(B, 30)
    x2[:, 30] = 1.0
    wcp = wc[:, 0:D]
    g = (wcp @ wcp.T).astype(np.float32)
    gwm = np.concatenate([g, wc], axis=1).astype(np.float16)
    return x2.astype(np.float16), gwm


def kernel(x, A, W):
    x2f, gwm = _host_prep(x, A, W)
    nc = _get_nc()
    in_maps = [
        {"xt": np.ascontiguousarray(np.roll(x2f, -QL * i, axis=0).T),
         "gw": gwm}
        for i in range(NCORES)
    ]
    res = run_bass_kernel_spmd(nc, in_maps, core_ids=list(range(NCORES)))
    return np.concatenate([res.results[i]["v"] for i in range(NCORES)], axis=0)


# revision 19
# speedup vs baseline: 1.0423x; 1.0307x over previous
"""GCN + batch-attention kernel for Trainium2 (8 NeuronCores, SPMD).

Problem (nn_GCNResnet): for x [8192,3,10], A [3,3], W [10,10]:
    adj   = 0.25*(off_diag_ones + A)                    # normalized adjacency
    pooled= 0.5*(h0+h1),  h = adj @ (x @ W)             # -> [B,10]
    v     = softmax(pooled @ pooled.T) @ pooled         # -> [B,10]

Formulation: pooled = x2 @ Wcp with x2 = x.reshape(B,30) padded to 32 features
(col 30 = ones, col 31 = 0) and Wcp the host-folded [32,10] weight. Scores
s_ij = pooled_i . pooled_j = x2_i^T G x2_j with G = Wcp Wcp^T [32,32], so the
kernel never materializes pooledT: S^T chunks come straight from x2T slices
(lhsT) against Gx2 = G @ x2T_local (rhs).

Host prep (pure layout/folding, no attention FLOPs): x2 is rolled per core,
cast to fp16 and sent already transposed as xt [32,8192]; Wc [32,12] has a
ones-selector col 10 (augmented-V ones column -> softmax denominator) and a
zero col 11; G is fp16.

Per core i (batch-sharded attention; input rolled by 1024*i rows so the
identical SPMD program always works on local query rows 0:1024):
  - Gx2 [32,1024] = G @ xt[:,0:1024] on PE, copied to SBUF fp16 (halves, so
    the first S matmul starts after half a copy)
  - vn [128,8g+j,12] = natural [pooled|1|0] rows, one 8-matmul group + one
    [128,96] copy per 1024-row group, interleaved with the chunk stream
  - flash attention, never materializing the [B,B] score matrix: for each of
    64 kv chunks, S^T [128,1024] = xt_chunk^T @ Gx2 (2 fp16 matmuls, fp32
    PSUM), then exp:
      * ACT chunks: native Exp -> fp16 E
      * DVE chunks: Schraudolph bit-trick in one tensor_scalar:
        int16(s*2^10/ln2 + (15*2^10 - 50)) bitcast to fp16 (~1% err; softmax
        normalization cancels most of it -- end-to-end ~2e-3 rel err)
    The ACT/DVE split is tuned so both engines drain the PSUM score stream
    in parallel (PSUM egress is the throughput wall: only ACT+DVE can read
    PSUM -- gpsimd and DMA cannot).
  - PV with E stationary: per chunk 8 matmuls out[128,12] += E[:,u::8]^T @
    vn_chunk accumulate in PSUM over all 64 chunks (N=12 fp16 at full rate,
    ~24x cheaper on PE than the vn-stationary orientation). The u::8 column
    stride makes partition p own query rows 8p..8p+7, so the output DMA
    writes one contiguous 320B line per partition.
  - epilogue: v = acc[:,0:10] * (1/acc[:,10]) on DVE, single DMA out.
"""

import math

import numpy as np

import concourse.bass as bass
import concourse.bacc as bacc
import concourse.mybir as mybir
import concourse.tile as tile
from concourse.bass_utils import run_bass_kernel_spmd

B = 8192
NCORES = 8
QL = B // NCORES          # 1024 local query rows
NF = 32                   # 30 feats + ones + zero pad
D = 10
DV = 12                   # [pooled | 1 | 0]
NG = 8                    # groups of 1024 batch rows
NKV = B // 128            # 64 kv chunks

f32 = mybir.dt.float32
f16 = mybir.dt.float16
i16 = mybir.dt.int16
EXP = mybir.ActivationFunctionType.Exp

# fp16 Schraudolph: exp(s) ~= bitcast_f16(int16(s*2^10/ln2 + 15*2^10 - CC));
# CC centers the piecewise-linear 2^frac error (tuned end-to-end on host).
C1 = 1024.0 / math.log(2.0)
CC = 50.0
C2 = 15.0 * 1024.0 - CC

N_DVE = 32                # kv chunks on the DVE bit-trick path (rest: ACT)

_NC = None


def _dve_mask():
    m = [((c + 1) * N_DVE) // NKV > (c * N_DVE) // NKV for c in range(NKV)]
    # drain: last chunk on ACT (shorter op), second-to-last on DVE
    if m[NKV - 1]:
        m[NKV - 1], m[NKV - 2] = False, True
    return m


def _build():
    nc = bacc.Bacc(trn_type="TRN2", target_bir_lowering=False)

    xt = nc.dram_tensor("xt", [NF, B], f16, kind="ExternalInput")
    gw = nc.dram_tensor("gw", [NF, NF + DV], f16, kind="ExternalInput")
    v = nc.dram_tensor("v", [QL, D], f32, kind="ExternalOutput")

    dve = _dve_mask()

    with tile.TileContext(nc) as tc:
        with (
            tc.tile_pool(name="const", bufs=1) as const,
            tc.tile_pool(name="xts", bufs=1) as xts,
            tc.tile_pool(name="vns", bufs=1) as vns,
            tc.tile_pool(name="epool", bufs=5) as epool,
            tc.tile_pool(name="outp", bufs=1) as outp,
            tc.tile_pool(name="psb", bufs=3, space="PSUM") as psb,
            tc.tile_pool(name="psv", bufs=1, space="PSUM") as psv,
            tc.tile_pool(name="psp", bufs=1, space="PSUM") as psp,
        ):
            gw_sb = const.tile([NF, NF + DV], f16, tag="gw")
            g_sb = gw_sb[:, 0:NF]
            wc_sb = gw_sb[:, NF:NF + DV]
            xt_sb = xts.tile([NF, B], f16, tag="xt")
            gx = const.tile([NF, QL], f16, tag="gx")
            vn = vns.tile([128, NKV, DV], f16, tag="vn")

            # gw + first local xt half on SP, second half on the ACT hwdge
            # queue: the input chains to the first S matmul resolve in
            # parallel at ~2.6us
            nc.sync.dma_start(out=gw_sb[:, :], in_=gw[:, :])
            nc.sync.dma_start(out=xt_sb[:, 0:512], in_=xt[:, 0:512])
            nc.sync.dma_start(out=xt_sb[:, 512:QL], in_=xt[:, 512:QL])

            # PE warm-up with no DMA dependency (starts the p-state ramp
            # clock). No dummy ACT op: the Exp table load auto-inserts before
            # the first real exp, and the xt0b DMA above must head the ACT
            # queue -- a dummy exp would drag the 1283ns table load to t=200
            # and delay that DMA past it.
            wz = const.tile([128, 128], f32, tag="wz")
            nc.vector.memset(wz[:, :], 0.0)
            warm = psb.tile([128, 1024], f32, tag="ps")
            for _ in range(2):
                nc.tensor.matmul(
                    warm[:, 0:64], wz[:, :], wz[:, 0:64],
                    start=True, stop=True,
                )

            for g in range(1, NG):
                nc.sync.dma_start(
                    out=xt_sb[:, QL * g:QL * (g + 1)],
                    in_=xt[:, QL * g:QL * (g + 1)])

            # Gx2 = G @ xt_local, half at a time so S(c0,h0) starts early
            gps = psb.tile([NF, QL], f32, tag="ps")
            for h in range(2):
                nc.tensor.matmul(
                    gps[:, 512 * h:512 * (h + 1)], g_sb,
                    xt_sb[:, 512 * h:512 * (h + 1)], start=True, stop=True,
                )
                nc.vector.tensor_copy(
                    gx[:, 512 * h:512 * (h + 1)],
                    gps[:, 512 * h:512 * (h + 1)])

            # double-buffered vn staging inside ONE psum bank: halves alternate
            # by group parity, so PE's vn(g+1) never waits on the copy of
            # vn(g) (a bufs=1 pool slot would serialize PE behind DVE here).
            vnp = psv.tile([128, 2, NG, DV], f32, tag="psv")

            def emit_vn(g):
                h = vnp[:, g % 2, :, :]
                for j in range(NG):
                    nc.tensor.matmul(
                        h[:, j, :],
                        xt_sb[:, QL * g + 128 * j:QL * g + 128 * (j + 1)],
                        wc_sb, start=(j == 0), stop=(j == NG - 1),
                    )
                nc.vector.tensor_copy(vn[:, NG * g:NG * (g + 1), :], h)

            def emit_s(c):
                st = psb.tile([128, QL], f32, tag="ps")
                lhs = xt_sb[:, 128 * c:128 * (c + 1)]
                for h in range(2):
                    nc.tensor.matmul(
                        st[:, 512 * h:512 * (h + 1)], lhs,
                        gx[:, 512 * h:512 * (h + 1)], start=True, stop=True,
                    )
                return st

            def emit_exp(c, st):
                et = epool.tile([128, QL], i16, tag="E")
                ev = et[:, :].bitcast(f16)
                if dve[c]:
                    nc.vector.tensor_scalar(
                        out=et[:, :], in0=st[:, :], scalar1=C1, scalar2=C2,
                        op0=mybir.AluOpType.mult, op1=mybir.AluOpType.add)
                else:
                    nc.scalar.activation(out=ev[:, :], in_=st[:, :], func=EXP)
                return ev

            pv = psp.tile([128, NG, DV], f32, tag="psp")

            def emit_pv(c, ev):
                rhs = vn[:, c, :]
                for u in range(NG):
                    lhs = bass.AP(ev.tensor, ev.offset + u, [ev.ap[0], [8, 128]])
                    nc.tensor.matmul(
                        pv[:, u, :], lhs, rhs,
                        start=(c == 0 and u == 0),
                        stop=(c == NKV - 1 and u == NG - 1),
                    )

            # chunk stream; vn(g) one group ahead at c%8==6; PV trails exp by
            # 2 chunks so PE's waiting PV matmuls never block S production.
            LOOK = 3
            e_tiles = {}
            s0 = emit_s(0)
            e_tiles[0] = emit_exp(0, s0)
            for c in range(1, NKV):
                if c == 2:
                    emit_vn(0)  # late enough not to preempt the gx copies
                g_next = c // NG + 1
                if c % NG == 6 and g_next < NG:
                    emit_vn(g_next)
                st = emit_s(c)
                e_tiles[c] = emit_exp(c, st)
                if c - LOOK >= 1:
                    emit_pv(c - LOOK, e_tiles.pop(c - LOOK))
                elif c == LOOK:
                    emit_pv(0, e_tiles.pop(0))
            for c in range(NKV - LOOK, NKV):
                emit_pv(c, e_tiles.pop(c))

            # ---- epilogue: v = acc[:,0:10] / acc[:,10] ----
            rec = outp.tile([128, NG], f32, tag="rec")
            nc.vector.reciprocal(rec[:, :], pv[:, :, D])
            vout = outp.tile([128, NG, D], f32, tag="vout")
            rec_b = bass.AP(rec[:, :].tensor, rec[:, :].offset,
                            [rec[:, :].ap[0], [1, NG], [0, D]])
            nc.vector.tensor_mul(vout[:, :, :], pv[:, :, 0:D], rec_b)
            dst = bass.AP(v, 0, [[NG * D, 128], [1, NG * D]])
            nc.sync.dma_start(
                out=dst, in_=vout[:, :, :].rearrange("p j d -> p (j d)"))

    nc.finalize()
    return nc


def _get_nc():
    global _NC
    if _NC is None:
        _NC = _build()
    return _NC


def _host_prep(x, A, W):
    """Fold adjacency normalization + node pooling into Wc [32,12]; build
    padded x2 [B,32] (col 30 = ones) and the score kernel G = Wcp Wcp^T."""
    A = np.asarray(A, np.float32)
    W = np.asarray(W, np.float32)
    off = np.ones((3, 3), np.float32) - np.eye(3, dtype=np.float32)
    d = 0.5 * np.eye(3, dtype=np.float32)
    adj = (d @ (off + A) @ d).astype(np.float32)
    c = (0.5 * (adj[0, :] + adj[1, :])).astype(np.float32)
    wc = np.zeros((NF, DV), np.float32)
    wc[0:30, 0:D] = np.einsum("n,fo->nfo", c, W).reshape(30, D)
    wc[30, D] = 1.0
    x2 = np.zeros((B, NF), np.float32)
    x2[:, 0:30] = np.asarray(x, np.float32).reshape(B, 30)
    x2[:, 30] = 1.0
    wcp = wc[:, 0:D]
    g = (wcp @ wcp.T).astype(np.float32)
    gwm = np.concatenate([g, wc], axis=1).astype(np.float16)
    return x2.astype(np.float16), gwm


def kernel(x, A, W):
    x2f, gwm = _host_prep(x, A, W)
    nc = _get_nc()
    in_maps = [
        {"xt": np.ascontiguousarray(np.roll(x2f, -QL * i, axis=0).T),
         "gw": gwm}
        for i in range(NCORES)
    ]
    res = run_bass_kernel_spmd(nc, in_maps, core_ids=list(range(NCORES)))
    return np.concatenate([res.results[i]["v"] for i in range(NCORES)], axis=0)
